# revision 27
# baseline (speedup 1.0000x reference)
"""Trainium2 Bass kernel: LayerNorm -> top-1 MoE -> v = clip(moe @ proj_w + b, +-3)
-> tridiagonal Green's-function diagonal via chunked Mobius scan
-> out = moe + bk*(spec @ out_w + out_b).

Sharding: data-parallel over flattened tokens (B*N = 8192) across 8 cores, 1024
tokens each (cores 2b/2b+1 own the halves of batch row b).

Top-1 routing is resolved on the HOST (fp32 LN+gate+argmax in numpy, matching
the reference's fp32 argmax): each core's tokens are sorted by expert into
per-expert column groups padded to 128-token tiles (group widths uniform
across cores => single SPMD program).  The device evaluates ONLY the selected
expert per token (~11/32 of the dense matmul work).  Per-token top softmax
prob is computed on device (1/sum(exp)).

Layout plumbing (all data-dependent indices are INPUTS, so the single SPMD
program serves all cores):
 - per-slot v is scattered to natural sequence order with per-tile indirect
   DMAs (overlapped with later groups' matmuls), pair-AllGathered, scanned.
 - G is written to DRAM packed (re,im) contiguously and gathered per slot
   with indirect DMAs.
 - weight/x/const DMAs are batched into few large transfers (the Sync
   sequencer costs ~0.6us per dma_start dispatch).
"""
import numpy as np
import ml_dtypes
_BF16NP = ml_dtypes.bfloat16
import concourse.bacc as bacc
import concourse.bass as bass
import concourse.mybir as mybir
from concourse.tile import TileContext
from concourse.bass_utils import run_bass_kernel_spmd
from concourse.alu_op_type import AluOpType

F32 = mybir.dt.float32
I32 = mybir.dt.int32
BF16 = mybir.dt.bfloat16
AF = mybir.ActivationFunctionType
AX = mybir.AxisListType
MULT, ADD, SUB = AluOpType.mult, AluOpType.add, AluOpType.subtract
MAXOP, MINOP = AluOpType.max, AluOpType.min

B, N, D, E = 4, 2048, 512, 4
H = 4 * D
P = 128
T = 1024          # real tokens per core
NCORE = 8

# cstA column offsets
CA_GAMMA, CA_BETA, CA_PROJW = 0, 512, 1024
CA_B2B, CA_B1C, CA_GWS, CA_GATEB, CA_IDM = 1536, 3584, 3648, 3664, 3668
CA_W = 3796
# cstB column offsets
CB_SHT, CB_JMAT, CB_W0, CB_W1O, CB_OUTB = 0, 896, 1024, 1536, 2048
CB_W = 2560


def build(proj_b_imm, g, debug=False):
    K = int(sum(g))               # token tiles per core (padded slot space)
    TS = K * P                    # slots per core
    S = np.concatenate([[0], np.cumsum(g)]).astype(int)  # tile starts/group

    nc = bacc.Bacc()
    dt = nc.dram_tensor
    xs = dt("xs", [TS, D], F32, kind="ExternalInput")
    cstA = dt("cstA", [P, CA_W], F32, kind="ExternalInput")
    cstB = dt("cstB", [P, CB_W], F32, kind="ExternalInput")
    csti = dt("csti", [P, 2 * K], I32, kind="ExternalInput")
    w1f = dt("w1f", [E * D, H], BF16, kind="ExternalInput")
    w2f = dt("w2f", [E * H, D], BF16, kind="ExternalInput")
    out = dt("out", [TS, D], F32, kind="ExternalOutput")
    # indirect-DMA targets need offset-0 APs -> standalone internal tensors
    vscat = dt("vscat", [T + 1, 1], F32, kind="Internal")
    grdp = dt("grdp", [N, 2], F32, kind="Internal")

    with TileContext(nc) as tc:
        with (tc.tile_pool(name="cst", bufs=1) as cst,
              tc.tile_pool(name="big", bufs=1) as big,
              tc.tile_pool(name="ht2", bufs=1) as ht2,
              tc.tile_pool(name="ln", bufs=1) as ln,
              tc.tile_pool(name="str", bufs=2) as strm,
              tc.tile_pool(name="rot", bufs=2) as rot,
              tc.tile_pool(name="sml", bufs=4) as sml,
              tc.tile_pool(name="ps", bufs=8, space="PSUM") as psp,
              tc.tile_pool(name="dr", bufs=1, space="DRAM") as dr):

            def tt(o, a, b, op):
                nc.vector.tensor_tensor(out=o, in0=a, in1=b, op=op)

            def gtt(o, a, b, op):
                nc.gpsimd.tensor_tensor(out=o, in0=a, in1=b, op=op)

            def ts(o, a, s1, s2, op0, op1=None):
                if op1 is None:
                    nc.vector.tensor_scalar(out=o, in0=a, scalar1=s1,
                                            scalar2=None, op0=op0)
                else:
                    nc.vector.tensor_scalar(out=o, in0=a, scalar1=s1,
                                            scalar2=s2, op0=op0, op1=op1)

            def gts(o, a, s1, s2, op0, op1=None):
                if op1 is None:
                    nc.gpsimd.tensor_scalar(out=o, in0=a, scalar1=s1,
                                            scalar2=None, op0=op0)
                else:
                    nc.gpsimd.tensor_scalar(out=o, in0=a, scalar1=s1,
                                            scalar2=s2, op0=op0, op1=op1)

            def stt(o, a, s, b, op0, op1):
                nc.vector.scalar_tensor_tensor(out=o, in0=a, scalar=s, in1=b,
                                               op0=op0, op1=op1)

            def gstt(o, a, s, b, op0, op1):
                nc.gpsimd.scalar_tensor_tensor(out=o, in0=a, scalar=s, in1=b,
                                               op0=op0, op1=op1)

            def cp(o, a):
                nc.vector.tensor_copy(out=o, in_=a)

            def gcp(o, a):
                nc.gpsimd.tensor_copy(out=o, in_=a)

            # ---- x per group (one DMA each), then packed constants ----
            xbig = []
            for e in range(E):
                ge = int(g[e])
                if ge == 0:
                    xbig.append(None)
                    continue
                xb = big.tile([P, ge * D], F32, tag=f"xbig{e}", name=f"xbig{e}")
                nc.sync.dma_start(
                    out=xb.rearrange("p (q j) -> p q j", q=ge),
                    in_=xs[S[e] * P:(S[e] + ge) * P, :].rearrange(
                        "(q p) j -> p q j", p=P))
                xbig.append(xb)

            cA = cst.tile([P, CA_W], F32, tag="cA")
            nc.sync.dma_start(out=cA, in_=cstA[:])
            cB = cst.tile([P, CB_W], F32, tag="cB")
            nc.sync.dma_start(out=cB, in_=cstB[:])
            cI = cst.tile([P, 2 * K], I32, tag="cI")
            nc.sync.dma_start(out=cI, in_=csti[:])

            gammat = cA[:, CA_GAMMA:CA_GAMMA + D]
            betat = cA[:, CA_BETA:CA_BETA + D]
            projwt = cA[:, CA_PROJW:CA_PROJW + D]
            b2bt = cA[:, CA_B2B:CA_B2B + E * D]
            b1ct = cA[:, CA_B1C:CA_B1C + 64]
            gwst = cA[:, CA_GWS:CA_GWS + 16]
            gatebt = cA[:, CA_GATEB:CA_GATEB + E]
            idt = cA[:, CA_IDM:CA_IDM + P]
            sht = cB[:, CB_SHT:CB_SHT + 7 * P]
            jmt = cB[:, CB_JMAT:CB_JMAT + P]
            w0t = cB[:, CB_W0:CB_W0 + D]
            w1ot = cB[:, CB_W1O:CB_W1O + D]
            outbt = cB[:, CB_OUTB:CB_OUTB + D]
            vsixt = cI[:, 0:K]
            ggixt = cI[:, K:2 * K]

            # warm-up collective: establish CC channels early
            wup = dr.tile([16], F32, name="wup", tag="wup")
            wug = dr.tile([32], F32, name="wug", tag="wug")
            wuz = sml.tile([1, 16], F32, tag="wuz")
            nc.vector.memset(wuz[:], 0.0)
            nc.sync.dma_start(out=wup[0:16], in_=wuz[0:1, :])
            nc.gpsimd.collective_compute(
                "AllGather", AluOpType.bypass,
                replica_groups=[[0, 1], [2, 3], [4, 5], [6, 7]],
                ins=[wup.opt()], outs=[wug.opt()])

            # per-group transposed activations (fp32 for gate, bf16 for mm1)
            xnTf = []
            xnTr = []
            for e in range(E):
                W = 128 * int(g[e])
                if W == 0:
                    xnTf.append(None)
                    xnTr.append(None)
                    continue
                xnTf.append(big.tile([P, 4 * W], F32, tag=f"xnTf{e}",
                                     name=f"xnTf{e}"))
                xnTr.append(big.tile([P, 4 * W], BF16, tag=f"xnTr{e}",
                                     name=f"xnTr{e}"))
            moe = big.tile([P, K * D], F32, tag="moe")   # = moe + out_b*bk
            ptall = sml.tile([P, K], F32, tag="ptall")
            vsb = sml.tile([P, K], F32, tag="vsb")
            gslot = sml.tile([P, 2 * K], F32, tag="gslot")

            vrow = dr.tile([2 * (T + 1)], F32, name="vrow", tag="vrow")

            IOX = bass.IndirectOffsetOnAxis

            # ========== fused stage A (LN/gate/transpose) + stage B per group
            for e in range(E):
                ge = int(g[e])
                if ge == 0:
                    continue
                W = 128 * ge
                # ---- stage A for this group's tiles ----
                for q in range(ge):
                    gt = S[e] + q
                    xt = xbig[e][:, q * D:(q + 1) * D]
                    musum = sml.tile([P, 1], F32, tag="musum")
                    nc.vector.tensor_reduce(out=musum, in_=xt, axis=AX.X,
                                            op=ADD)
                    mu = sml.tile([P, 1], F32, tag="mu")
                    ts(mu, musum, 1.0 / D, None, MULT)
                    mneg = sml.tile([P, 1], F32, tag="mnegl")
                    ts(mneg, musum, -1.0 / D, None, MULT)
                    scr = ln.tile([P, D], F32, tag="scr")
                    varsum = sml.tile([P, 1], F32, tag="varsum")
                    nc.scalar.activation(out=scr, in_=xt, func=AF.Square,
                                         bias=mneg[:, 0:1], scale=1.0,
                                         accum_out=varsum[:, 0:1])
                    vtmp = sml.tile([P, 1], F32, tag="vtmp")
                    ts(vtmp, varsum, 1.0 / D, 1e-5, MULT, ADD)
                    vsq = sml.tile([P, 1], F32, tag="vsq")
                    nc.scalar.activation(out=vsq, in_=vtmp, func=AF.Sqrt)
                    rstd = sml.tile([P, 1], F32, tag="rstd")
                    nc.vector.reciprocal(out=rstd, in_=vsq)
                    xn = ln.tile([P, D], F32, tag="xn")
                    ts(xn, xt, mu[:, 0:1], rstd[:, 0:1], SUB, MULT)
                    xng = ln.tile([P, D], F32, tag="xng")
                    tt(xng, xn, gammat, MULT)
                    xnf = ln.tile([P, D], F32, tag="xnf")
                    tt(xnf, xng, betat, ADD)
                    for db in range(4):
                        pst = psp.tile([P, P], F32, tag="mm")
                        nc.tensor.transpose(pst[:], xnf[:, db * P:(db + 1) * P],
                                            idt)
                        cp(xnTf[e][:, db * W + q * P: db * W + (q + 1) * P],
                           pst[:])
                    # gate logits -> top prob (softmax denominator reciprocal)
                    psg = psp.tile([P, E], F32, tag="mm")
                    for db in range(4):
                        nc.tensor.matmul(
                            out=psg,
                            lhsT=xnTf[e][:, db * W + q * P: db * W + (q + 1) * P],
                            rhs=gwst[:, db * E:(db + 1) * E],
                            start=(db == 0), stop=(db == 3))
                    lg = sml.tile([P, E], F32, tag="lg")
                    stt(lg, psg, 1.0, gatebt, MULT, ADD)
                    mx = sml.tile([P, 1], F32, tag="mx")
                    nc.vector.tensor_reduce(out=mx, in_=lg, axis=AX.X, op=MAXOP)
                    mxneg = sml.tile([P, 1], F32, tag="mxneg")
                    ts(mxneg, mx, -1.0, None, MULT)
                    el = sml.tile([P, E], F32, tag="el")
                    ssum = sml.tile([P, 1], F32, tag="ssum")
                    nc.scalar.activation(out=el, in_=lg, func=AF.Exp,
                                         bias=mxneg[:, 0:1], scale=1.0,
                                         accum_out=ssum[:, 0:1])
                    nc.vector.reciprocal(out=ptall[:, gt:gt + 1], in_=ssum)
                    xv_in = xnTf[e].rearrange("p (db t) -> p db t", db=4)[
                        :, :, q * P:(q + 1) * P]
                    xv_out = xnTr[e].rearrange("p (db t) -> p db t", db=4)[
                        :, :, q * P:(q + 1) * P]
                    gcp(xv_out, xv_in)

                # ---- stage B: selected-expert MoE for this group ----
                # weights fetched in 4-hc blocks (one dma_start per block)
                hT = ht2.tile([P, 16 * 512], BF16, tag=f"hT{e % 2}",
                              name=f"hT{e}")
                for hb in range(4):
                    w1blk = strm.tile([P, 4 * D], BF16, tag="w1blk")
                    nc.sync.dma_start(
                        out=w1blk.rearrange("p (db j) -> p db j", db=4),
                        in_=w1f[e * D:(e + 1) * D,
                                hb * 4 * P:(hb + 1) * 4 * P].rearrange(
                                    "(db p) j -> p db j", p=P))
                    for hq in range(4):
                        hc = hb * 4 + hq
                        psh = psp.tile([P, W], F32, tag="mm",
                                       name=f"psh{e}_{hc}")
                        for db in range(4):
                            nc.tensor.matmul(
                                out=psh,
                                lhsT=w1blk[:, db * 4 * P + hq * P:
                                           db * 4 * P + (hq + 1) * P],
                                rhs=xnTr[e][:, db * W:(db + 1) * W],
                                start=(db == 0), stop=(db == 3))
                        nc.scalar.activation(
                            out=hT[:, hc * W:(hc + 1) * W],
                            in_=psh, func=AF.Gelu_apprx_tanh,
                            bias=b1ct[:, e * 16 + hc: e * 16 + hc + 1],
                            scale=1.0)
                pso = [psp.tile([P, D], F32, tag="mm", name=f"pso{e}_{i}")
                       for i in range(ge)]
                for hb in range(4):
                    w2blk = strm.tile([P, 4 * D], BF16, tag="w2blk")
                    nc.sync.dma_start(
                        out=w2blk.rearrange("p (hc j) -> p hc j", hc=4),
                        in_=w2f[e * H + hb * 4 * P:
                                e * H + (hb + 1) * 4 * P, :].rearrange(
                                    "(hc p) j -> p hc j", p=P))
                    for hq in range(4):
                        hc = hb * 4 + hq
                        for q in range(ge):
                            nc.tensor.matmul(
                                out=pso[q],
                                lhsT=hT[:, hc * W + q * P: hc * W + (q + 1) * P],
                                rhs=w2blk[:, hq * D:(hq + 1) * D],
                                start=(hc == 0), stop=(hc == 15))
                for q in range(ge):
                    gt = S[e] + q
                    mslice = moe[:, gt * D:(gt + 1) * D]
                    pt_ap = ptall[:, gt:gt + 1]
                    if gt % 2 == 0:
                        eo = rot.tile([P, D], F32, tag="wrk")
                        stt(eo, pso[q], 1.0, b2bt[:, e * D:(e + 1) * D],
                            MULT, ADD)
                        # v from eo: (sum(eo*projw))*ptop + proj_b, clipped
                        tv = rot.tile([P, D], F32, tag="wrk")
                        tt(tv, eo, projwt, MULT)
                        vs = sml.tile([P, 1], F32, tag="vs")
                        nc.vector.tensor_reduce(out=vs, in_=tv,
                                                axis=AX.X, op=ADD)
                        vt1 = sml.tile([P, 1], F32, tag="vt1")
                        ts(vt1, vs, pt_ap, proj_b_imm, MULT, ADD)
                        ts(vsb[:, gt:gt + 1], vt1, 3.0, -3.0, MINOP, MAXOP)
                        # moe slice with out_b*bk folded in
                        stt(mslice, eo, pt_ap, outbt, MULT, ADD)
                    else:
                        eo = rot.tile([P, D], F32, tag="gwrk")
                        stt(eo, pso[q], 1.0, b2bt[:, e * D:(e + 1) * D],
                            MULT, ADD)
                        tv = rot.tile([P, D], F32, tag="gwrk")
                        gtt(tv, eo, projwt, MULT)
                        scr2 = rot.tile([P, D], F32, tag="swrk")
                        vsg = sml.tile([P, 1], F32, tag="vsg")
                        nc.scalar.activation(out=scr2, in_=tv,
                                             func=AF.Copy,
                                             accum_out=vsg[:, 0:1])
                        vt1g = sml.tile([P, 1], F32, tag="vt1g")
                        ts(vt1g, vsg, pt_ap, proj_b_imm, MULT, ADD)
                        ts(vsb[:, gt:gt + 1], vt1g, 3.0, -3.0, MINOP, MAXOP)
                        gtt(mslice, eo, pt_ap.broadcast_to([P, D]), MULT)
                        gtt(mslice, mslice, outbt, ADD)
                    # scatter this tile's v into natural positions now so the
                    # DMA overlaps the remaining groups' compute
                    nc.gpsimd.indirect_dma_start(
                        out=vscat[:], out_offset=IOX(ap=vsixt[:, gt:gt + 1],
                                                     axis=0),
                        in_=vsb[:, gt:gt + 1], in_offset=None)

            # ================= stage D: pair AllGather + scan inputs ==========
            nc.gpsimd.collective_compute(
                "AllGather", AluOpType.bypass,
                replica_groups=[[0, 1], [2, 3], [4, 5], [6, 7]],
                ins=[vscat[:].opt()], outs=[vrow.opt()])
            av = sml.tile([P, 16], F32, tag="av")
            nc.sync.dma_start(out=av[0:64, :], in_=vrow[0:T])
            nc.sync.dma_start(out=av[64:128, :], in_=vrow[T + 1:2 * T + 1])
            arf = sml.tile([P, 16], F32, tag="arf")
            ts(arf, av, -1.0, 2.0, MULT, ADD)        # a_re = 2 - v
            psj = psp.tile([P, 16], F32, tag="mm")
            nc.tensor.matmul(out=psj, lhsT=jmt, rhs=arf[:], start=True,
                             stop=True)
            arb = sml.tile([P, 16], F32, tag="arb")
            cp(arb, psj[:, 15::-1])                  # a_re reversed seq

            # ================= stage E: Mobius scan ===========================
            wfr = sml.tile([P, 36], F32, tag="wfr")
            wfi = sml.tile([P, 36], F32, tag="wfi")
            wbr = sml.tile([P, 36], F32, tag="wbr")
            wbi = sml.tile([P, 36], F32, tag="wbi")
            nc.vector.memset(wfr[:, 0:1], 0.0)
            nc.vector.memset(wfr[:, 1:2], 1.0)
            nc.vector.memset(wfr[:, 2:3], 1.0)
            nc.vector.memset(wfr[:, 3:4], 0.0)
            nc.vector.memset(wfi[:, 0:4], 0.0)
            nc.vector.memset(wbr[:, 0:1], 0.0)
            nc.vector.memset(wbr[:, 1:2], 1.0)
            nc.vector.memset(wbr[:, 2:3], 1.0)
            nc.vector.memset(wbr[:, 3:4], 0.0)
            nc.vector.memset(wbi[:, 0:4], 0.0)
            # L1 as 4 independent 8-step half-chains (fwd-L/fwd-R/bwd-L on
            # Vector with fused stt steps, bwd-R on GpSimd), then wide complex
            # merges: block_{8+k} = ru_k*B9 + rv_k*B8.
            rwfr = sml.tile([P, 20], F32, tag="rwfr")
            rwfi = sml.tile([P, 20], F32, tag="rwfi")
            rwbr = sml.tile([P, 20], F32, tag="rwbr")
            rwbi = sml.tile([P, 20], F32, tag="rwbi")
            nc.vector.memset(rwfr[:, 0:1], 0.0)
            nc.vector.memset(rwfr[:, 1:2], 1.0)
            nc.vector.memset(rwfr[:, 2:3], 1.0)
            nc.vector.memset(rwfr[:, 3:4], 0.0)
            nc.vector.memset(rwfi[:, 0:4], 0.0)
            nc.gpsimd.memset(rwbr[:, 0:1], 0.0)
            nc.gpsimd.memset(rwbr[:, 1:2], 1.0)
            nc.gpsimd.memset(rwbr[:, 2:3], 1.0)
            nc.gpsimd.memset(rwbr[:, 3:4], 0.0)
            nc.gpsimd.memset(rwbi[:, 0:4], 0.0)
            tfa = sml.tile([P, 2], F32, tag="taf")
            tfb = sml.tile([P, 2], F32, tag="tbf")
            tfc = sml.tile([P, 2], F32, tag="tfc")
            tfd = sml.tile([P, 2], F32, tag="tfd")
            tba = sml.tile([P, 2], F32, tag="tab")
            tbc = sml.tile([P, 2], F32, tag="tbc")
            tga = sml.tile([P, 2], F32, tag="tga")
            tgb = sml.tile([P, 2], F32, tag="tgb")
            for t in range(8):
                s0 = slice(2 * t, 2 * t + 2)
                s1 = slice(2 * t + 2, 2 * t + 4)
                s2 = slice(2 * t + 4, 2 * t + 6)
                afL = arf[:, t:t + 1]
                afR = arf[:, 8 + t:9 + t]
                abL = arb[:, t:t + 1]
                abR = arb[:, 8 + t:9 + t]
                stt(tfa, wfr[:, s1], afL, wfi[:, s1], MULT, SUB)
                tt(wfr[:, s2], tfa, wfr[:, s0], SUB)
                stt(tfb, wfi[:, s1], afL, wfr[:, s1], MULT, ADD)
                tt(wfi[:, s2], tfb, wfi[:, s0], SUB)
                stt(tfc, rwfr[:, s1], afR, rwfi[:, s1], MULT, SUB)
                tt(rwfr[:, s2], tfc, rwfr[:, s0], SUB)
                stt(tfd, rwfi[:, s1], afR, rwfr[:, s1], MULT, ADD)
                tt(rwfi[:, s2], tfd, rwfi[:, s0], SUB)
                stt(tba, wbr[:, s1], abL, wbi[:, s1], MULT, SUB)
                tt(wbr[:, s2], tba, wbr[:, s0], SUB)
                stt(tbc, wbi[:, s1], abL, wbr[:, s1], MULT, ADD)
                tt(wbi[:, s2], tbc, wbi[:, s0], SUB)
                gtt(tga, rwbr[:, s1], abR.broadcast_to([P, 2]), MULT)
                gtt(tgb, tga, rwbi[:, s1], SUB)
                gtt(rwbr[:, s2], tgb, rwbr[:, s0], SUB)
                gtt(tga, rwbi[:, s1], abR.broadcast_to([P, 2]), MULT)
                gtt(tgb, tga, rwbr[:, s1], ADD)
                gtt(rwbi[:, s2], tgb, rwbi[:, s0], SUB)

            def bview(tile36r, lo):
                return tile36r[:, lo:lo + 2].rearrange(
                    "p (one c) -> p one c", one=1).broadcast_to([P, 8, 2])

            def rview(tile20, c):
                return tile20[:, 4:20].rearrange(
                    "p (k c) -> p k c", c=2)[:, :, c:c + 1].broadcast_to(
                        [P, 8, 2])

            def pv16(tl):
                return tl.rearrange("p (k c) -> p k c", c=2)

            mg1 = sml.tile([P, 16], F32, tag="mg1")
            mg2 = sml.tile([P, 16], F32, tag="mg2")
            mg3 = sml.tile([P, 16], F32, tag="mg3")
            mg4 = sml.tile([P, 16], F32, tag="mg4")
            # fwd merge on Vector: blocks 10..17
            tt(pv16(mg1), rview(rwfr, 0), bview(wfr, 18), MULT)
            tt(pv16(mg2), rview(rwfi, 0), bview(wfi, 18), MULT)
            tt(pv16(mg3), rview(rwfr, 1), bview(wfr, 16), MULT)
            tt(pv16(mg4), rview(rwfi, 1), bview(wfi, 16), MULT)
            tt(mg1, mg1, mg2, SUB)
            tt(mg3, mg3, mg4, SUB)
            tt(wfr[:, 20:36], mg1, mg3, ADD)
            tt(pv16(mg1), rview(rwfr, 0), bview(wfi, 18), MULT)
            tt(pv16(mg2), rview(rwfi, 0), bview(wfr, 18), MULT)
            tt(pv16(mg3), rview(rwfr, 1), bview(wfi, 16), MULT)
            tt(pv16(mg4), rview(rwfi, 1), bview(wfr, 16), MULT)
            tt(mg1, mg1, mg2, ADD)
            tt(mg3, mg3, mg4, ADD)
            tt(wfi[:, 20:36], mg1, mg3, ADD)
            # bwd merge on GpSimd
            mg5 = sml.tile([P, 16], F32, tag="mg5")
            mg6 = sml.tile([P, 16], F32, tag="mg6")
            mg7 = sml.tile([P, 16], F32, tag="mg7")
            mg8 = sml.tile([P, 16], F32, tag="mg8")
            gtt(pv16(mg5), rview(rwbr, 0), bview(wbr, 18), MULT)
            gtt(pv16(mg6), rview(rwbi, 0), bview(wbi, 18), MULT)
            gtt(pv16(mg7), rview(rwbr, 1), bview(wbr, 16), MULT)
            gtt(pv16(mg8), rview(rwbi, 1), bview(wbi, 16), MULT)
            gtt(mg5, mg5, mg6, SUB)
            gtt(mg7, mg7, mg8, SUB)
            gtt(wbr[:, 20:36], mg5, mg7, ADD)
            gtt(pv16(mg5), rview(rwbr, 0), bview(wbi, 18), MULT)
            gtt(pv16(mg6), rview(rwbi, 0), bview(wbr, 18), MULT)
            gtt(pv16(mg7), rview(rwbr, 1), bview(wbi, 16), MULT)
            gtt(pv16(mg8), rview(rwbi, 1), bview(wbr, 16), MULT)
            gtt(mg5, mg5, mg6, ADD)
            gtt(mg7, mg7, mg8, ADD)
            gtt(wbi[:, 20:36], mg5, mg7, ADD)
            q = sml.tile([P, 16], F32, tag="qa")
            cp(q[:, 0:4:2], wfr[:, 34:36])      # m00, m01 = block 17
            cp(q[:, 4:8:2], wfr[:, 32:34])      # m10, m11 = block 16
            cp(q[:, 8:12:2], wfi[:, 34:36])
            cp(q[:, 12:16:2], wfi[:, 32:34])
            gcp(q[:, 1:4:2], wbr[:, 34:36])
            gcp(q[:, 5:8:2], wbr[:, 32:34])
            gcp(q[:, 9:12:2], wbi[:, 34:36])
            gcp(q[:, 13:16:2], wbi[:, 32:34])

            rn1 = sml.tile([P, 2], F32, tag="rn1")
            rn2 = sml.tile([P, 2], F32, tag="rn2")
            rn3 = sml.tile([P, 2], F32, tag="rn3")

            def renorm(qq):
                tt(rn1, qq[:, 0:2], qq[:, 0:2], MULT)
                tt(rn2, qq[:, 8:10], qq[:, 8:10], MULT)
                tt(rn3, rn1, rn2, ADD)
                nc.scalar.activation(out=rn1, in_=rn3, func=AF.Sqrt)
                nc.vector.reciprocal(out=rn2, in_=rn1)
                ts(qq[:, 0:16:2], qq[:, 0:16:2], rn2[:, 0:1], None, MULT)
                gtt(qq[:, 1:16:2], qq[:, 1:16:2],
                    rn2[:, 1:2].broadcast_to([P, 8]), MULT)

            renorm(q)

            p1t = sml.tile([P, 16], F32, tag="p1t")
            p2t = sml.tile([P, 16], F32, tag="p2t")
            p3t = sml.tile([P, 16], F32, tag="p3t")
            p4t = sml.tile([P, 16], F32, tag="p4t")
            crt = sml.tile([P, 16], F32, tag="crt")
            cit = sml.tile([P, 16], F32, tag="cit")

            def qa_v(tile16, ri, ii):
                v = tile16[:, 8 * ri + 4 * ii: 8 * ri + 4 * ii + 4]
                v = v.rearrange("p (k d) -> p k d", k=2, d=2)
                return v.unsqueeze(1).broadcast_to([P, 2, 2, 2])

            def qb_v(tile16, ri):
                v = tile16[:, 8 * ri: 8 * ri + 8]
                return v.rearrange("p (k j d) -> p j k d", k=2, j=2, d=2)

            def pv(tile16, ii):
                return tile16[:, 8 * ii: 8 * ii + 8].rearrange(
                    "p (j k d) -> p j k d", j=2, k=2, d=2)

            for i, s in enumerate((1, 2, 4, 8, 16, 32, 64)):
                psq = psp.tile([P, 16], F32, tag="mm")
                nc.tensor.matmul(out=psq, lhsT=sht[:, i * P:(i + 1) * P],
                                 rhs=q[:], start=True, stop=True)
                qs = sml.tile([P, 16], F32, tag=("qsa" if i % 2 == 0 else "qsb"))
                cp(qs, psq[:])
                nc.vector.memset(qs[0:s, 0:2], 1.0)   # identity pad m00
                nc.vector.memset(qs[0:s, 6:8], 1.0)   # identity pad m11
                qbr, qbi = qb_v(qs, 0), qb_v(qs, 1)
                for ii in range(2):
                    tt(pv(p1t, ii), qa_v(q, 0, ii), qbr, MULT)
                    tt(pv(p2t, ii), qa_v(q, 1, ii), qbi, MULT)
                    gtt(pv(p3t, ii), qa_v(q, 0, ii), qbi, MULT)
                    gtt(pv(p4t, ii), qa_v(q, 1, ii), qbr, MULT)
                tt(crt, p1t, p2t, SUB)
                gtt(cit, p3t, p4t, ADD)
                qn = sml.tile([P, 16], F32, tag=("qb" if i % 2 == 0 else "qa"))
                crv = crt.rearrange("p (ak d) -> p ak d", ak=8, d=2)
                civ = cit.rearrange("p (ak d) -> p ak d", ak=8, d=2)
                tt(qn[:, 0:8].rearrange("p (a d) -> p a d", a=4),
                   crv[:, 0:8:2, :], crv[:, 1:8:2, :], ADD)
                gtt(qn[:, 8:16].rearrange("p (a d) -> p a d", a=4),
                    civ[:, 0:8:2, :], civ[:, 1:8:2, :], ADD)
                q = qn
                if i in (2, 5):
                    renorm(q)

            # L3: interior values from the saved L1 partials
            psq1 = psp.tile([P, 16], F32, tag="mm")
            nc.tensor.matmul(out=psq1, lhsT=sht[:, 0:P], rhs=q[:],
                             start=True, stop=True)
            nc.vector.memset(psq1[0:1, 0:2], 1.0)      # chunk0 start x = 1
            sq1 = sml.tile([P, 16], F32, tag="sq1")
            cp(sq1, psq1[:])
            xsr_f, xsi_f = sq1[:, 0:1], sq1[:, 8:9]
            ysr_f, ysi_f = sq1[:, 4:5], sq1[:, 12:13]
            xsr_b, xsi_b = sq1[:, 1:2], sq1[:, 9:10]
            ysr_b, ysi_b = sq1[:, 5:6], sq1[:, 13:14]
            m00r, m00i = wfr[:, 2:36:2], wfi[:, 2:36:2]   # 17 blocks (1..17)
            m01r, m01i = wfr[:, 3:36:2], wfi[:, 3:36:2]
            n00r, n00i = wbr[:, 2:36:2], wbi[:, 2:36:2]
            n01r, n01i = wbr[:, 3:36:2], wbi[:, 3:36:2]
            ra = sml.tile([P, 17], F32, tag="ra")
            rb = sml.tile([P, 17], F32, tag="rb")
            rc = sml.tile([P, 17], F32, tag="rc")
            rd = sml.tile([P, 17], F32, tag="rd")
            gra = sml.tile([P, 17], F32, tag="gra")
            grb = sml.tile([P, 17], F32, tag="grb")
            grc = sml.tile([P, 17], F32, tag="grc")
            grd2 = sml.tile([P, 17], F32, tag="grd2")
            pxr = sml.tile([P, 17], F32, tag="pxr")
            pxi = sml.tile([P, 17], F32, tag="pxi")
            pbr = sml.tile([P, 17], F32, tag="pbr")
            pbi = sml.tile([P, 17], F32, tag="pbi")
            ts(ra, m00r, xsr_f, None, MULT)
            ts(rb, m00i, xsi_f, None, MULT)
            tt(rc, ra, rb, SUB)
            ts(ra, m01r, ysr_f, None, MULT)
            ts(rb, m01i, ysi_f, None, MULT)
            tt(rd, ra, rb, SUB)
            tt(pxr, rc, rd, ADD)
            ts(ra, m00r, xsi_f, None, MULT)
            ts(rb, m00i, xsr_f, None, MULT)
            tt(rc, ra, rb, ADD)
            ts(ra, m01r, ysi_f, None, MULT)
            ts(rb, m01i, ysr_f, None, MULT)
            tt(rd, ra, rb, ADD)
            tt(pxi, rc, rd, ADD)

            def bc17(apv):
                return apv.broadcast_to([P, 17])
            gtt(gra, n00r, bc17(xsr_b), MULT)
            gtt(grb, n00i, bc17(xsi_b), MULT)
            gtt(grc, gra, grb, SUB)
            gtt(gra, n01r, bc17(ysr_b), MULT)
            gtt(grb, n01i, bc17(ysi_b), MULT)
            gtt(grd2, gra, grb, SUB)
            gtt(pbr, grc, grd2, ADD)
            gtt(gra, n00r, bc17(xsi_b), MULT)
            gtt(grb, n00i, bc17(xsr_b), MULT)
            gtt(grc, gra, grb, ADD)
            gtt(gra, n01r, bc17(ysi_b), MULT)
            gtt(grb, n01i, bc17(ysr_b), MULT)
            gtt(grd2, gra, grb, ADD)
            gtt(pbi, grc, grd2, ADD)

            psfr = psp.tile([P, 17], F32, tag="mm")
            nc.tensor.matmul(out=psfr, lhsT=jmt, rhs=pbr[:], start=True,
                             stop=True)
            psfi = psp.tile([P, 17], F32, tag="mm")
            nc.tensor.matmul(out=psfi, lhsT=jmt, rhs=pbi[:], start=True,
                             stop=True)
            sfr = sml.tile([P, 17], F32, tag="sfr")
            sfi = sml.tile([P, 17], F32, tag="sfi")
            cp(sfr, psfr[:])
            cp(sfi, psfi[:])

            uxr, uxi = pxr[:, 1:17], pxi[:, 1:17]
            uyr, uyi = pxr[:, 0:16], pxi[:, 0:16]
            wxr, wxi = sfr[:, 16:0:-1], sfi[:, 16:0:-1]
            wyr, wyi = sfr[:, 15::-1], sfi[:, 15::-1]

            def ctile(tag):
                return sml.tile([P, 16], F32, tag=tag, name=tag)

            sa, sb = ctile("sa"), ctile("sb")
            ga, gb = ctile("ga"), ctile("gb")
            nr_, ni_ = ctile("nr"), ctile("ni")
            t1r, t1i = ctile("t1r"), ctile("t1i")
            t2r, t2i = ctile("t2r"), ctile("t2i")
            t3r, t3i = ctile("t3r"), ctile("t3i")
            drt, dit = ctile("drt"), ctile("dit")
            magt, invt = ctile("magt"), ctile("invt")
            gr, gi = ctile("gr"), ctile("gi")

            def cmul(or_, oi_, xr_, xi_, yr_, yi_):
                tt(sa, xr_, yr_, MULT)
                tt(sb, xi_, yi_, MULT)
                tt(or_, sa, sb, SUB)
                tt(sa, xr_, yi_, MULT)
                tt(sb, xi_, yr_, MULT)
                tt(oi_, sa, sb, ADD)

            def gcmul(or_, oi_, xr_, xi_, yr_, yi_):
                gtt(ga, xr_, yr_, MULT)
                gtt(gb, xi_, yi_, MULT)
                gtt(or_, ga, gb, SUB)
                gtt(ga, xr_, yi_, MULT)
                gtt(gb, xi_, yr_, MULT)
                gtt(oi_, ga, gb, ADD)

            cmul(nr_, ni_, uyr, uyi, wyr, wyi)       # num = Uy*Wy
            gcmul(t1r, t1i, uxr, uxi, wyr, wyi)
            cmul(t2r, t2i, wxr, wxi, uyr, uyi)
            gtt(ga, arf, nr_, MULT)                  # t3 = a*num, a = arf + 1j
            gtt(t3r, ga, ni_, SUB)
            gtt(gb, arf, ni_, MULT)
            gtt(t3i, gb, nr_, ADD)
            tt(sa, t1r, t2r, ADD)
            tt(drt, sa, t3r, SUB)
            tt(sb, t1i, t2i, ADD)
            tt(dit, sb, t3i, SUB)
            tt(sa, drt, drt, MULT)
            tt(sb, dit, dit, MULT)
            tt(magt, sa, sb, ADD)
            nc.vector.reciprocal(out=invt, in_=magt)
            tt(sa, nr_, drt, MULT)
            tt(sb, ni_, dit, MULT)
            tt(gr, sa, sb, ADD)
            tt(gr, gr, invt, MULT)
            gtt(ga, ni_, drt, MULT)
            gtt(gb, nr_, dit, MULT)
            gtt(gi, ga, gb, SUB)
            gtt(gi, gi, invt, MULT)

            # ============ stage F: G -> packed DRAM (contiguous) -> gather ====
            gpair = sml.tile([P, 32], F32, tag="gpair")
            cp(gpair.rearrange("p (j two) -> p j two", two=2)[:, :, 0], gr[:])
            cp(gpair.rearrange("p (j two) -> p j two", two=2)[:, :, 1], gi[:])
            nc.sync.dma_start(
                out=grdp[:, :].rearrange("(p j) two -> p (j two)", p=P),
                in_=gpair[:])
            for t in range(K):
                nc.gpsimd.indirect_dma_start(
                    out=gslot[:, 2 * t:2 * t + 2], out_offset=None,
                    in_=grdp[:],
                    in_offset=IOX(ap=ggixt[:, t:t + 1], axis=0))

            # ================= stage G: final combine (2 ops/tile) ============
            for t in range(K):
                acc = rot.tile([P, D], F32, tag="wrk")
                stt(acc, w0t, gslot[:, 2 * t:2 * t + 1],
                    moe[:, t * D:(t + 1) * D], MULT, ADD)
                ott = rot.tile([P, D], F32, tag="wrk")
                stt(ott, w1ot, gslot[:, 2 * t + 1:2 * t + 2], acc, MULT, ADD)
                nc.sync.dma_start(out=out[t * P:(t + 1) * P, :], in_=ott)
    nc.finalize()
    return nc


def _prep_inputs(inputs):
    f = np.float32
    x = np.ascontiguousarray(np.asarray(inputs["x"], f).reshape(B * N, D))
    gamma = np.asarray(inputs["ln_gamma"], f)
    beta = np.asarray(inputs["ln_beta"], f)
    gate_w = np.asarray(inputs["gate_w"], f)
    gate_b = np.asarray(inputs["gate_b"], f)
    w1 = np.asarray(inputs["w1"], f)
    b1 = np.asarray(inputs["b1"], f)
    w2 = np.asarray(inputs["w2"], f)
    b2 = np.asarray(inputs["b2"], f)
    proj_w = np.asarray(inputs["proj_w"], f)[:, 0]
    out_w = np.asarray(inputs["out_w"], f)
    out_b = np.asarray(inputs["out_b"], f)
    bk = f(np.asarray(inputs["bk_scale"], f).reshape(-1)[0])

    # ---- host routing (argmax of gate logits over LN'd x, fp32) ----
    mu = x.mean(-1, keepdims=True, dtype=np.float32)
    xc = x - mu
    var = np.mean(xc * xc, axis=-1, keepdims=True, dtype=np.float32)
    xn = xc / np.sqrt(var + np.float32(1e-5)) * gamma + beta
    lg = xn @ gate_w + gate_b
    eid = lg.argmax(-1)                                    # (8192,)

    cnt = np.zeros((NCORE, E), np.int64)
    for c in range(NCORE):
        cnt[c] = np.bincount(eid[c * T:(c + 1) * T], minlength=E)
    g = np.ceil(cnt.max(axis=0) / P).astype(int)           # tiles per expert
    K = int(g.sum())
    TS = K * P
    S = np.concatenate([[0], np.cumsum(g)]).astype(int)

    def bcast(v, w):
        return np.ascontiguousarray(np.broadcast_to(v.astype(f), (P, w)))

    cstA = np.zeros((P, CA_W), f)
    cstA[:, CA_GAMMA:CA_GAMMA + D] = bcast(gamma, D)
    cstA[:, CA_BETA:CA_BETA + D] = bcast(beta, D)
    cstA[:, CA_PROJW:CA_PROJW + D] = bcast(proj_w, D)
    cstA[:, CA_B2B:CA_B2B + E * D] = bcast(b2.reshape(E * D), E * D)
    cstA[:, CA_B1C:CA_B1C + 64] = np.ascontiguousarray(
        b1.reshape(E, 16, P).transpose(2, 0, 1).reshape(P, 64))
    cstA[:, CA_GWS:CA_GWS + 16] = np.ascontiguousarray(
        gate_w.reshape(4, P, E).transpose(1, 0, 2).reshape(P, 16))
    cstA[:, CA_GATEB:CA_GATEB + E] = bcast(gate_b, E)
    cstA[:, CA_IDM:CA_IDM + P] = np.eye(P, dtype=f)

    cstB = np.zeros((P, CB_W), f)
    cstB[:, CB_SHT:CB_SHT + 7 * P] = np.concatenate(
        [np.eye(P, k=s, dtype=f) for s in (1, 2, 4, 8, 16, 32, 64)], axis=1)
    cstB[:, CB_JMAT:CB_JMAT + P] = np.eye(P, dtype=f)[::-1]
    cstB[:, CB_W0:CB_W0 + D] = bcast(out_w[0] * bk, D)
    cstB[:, CB_W1O:CB_W1O + D] = bcast(out_w[1] * bk, D)
    cstB[:, CB_OUTB:CB_OUTB + D] = bcast(out_b * bk, D)

    common = dict(
        cstA=cstA,
        cstB=cstB,
        w1f=np.ascontiguousarray(w1.reshape(E * D, H)).astype(_BF16NP),
        w2f=np.ascontiguousarray(w2.reshape(E * H, D)).astype(_BF16NP),
    )
    in_maps = []
    slotnat = []                     # per core: slot -> local natural (-1 pad)
    for c in range(NCORE):
        ec = eid[c * T:(c + 1) * T]
        perm = np.full(TS, -1, np.int64)
        slots_nat = np.full(TS, -1, np.int64)
        for e in range(E):
            ids = np.nonzero(ec == e)[0]
            if g[e] == 0:
                continue
            s0 = S[e] * P
            perm[s0:s0 + len(ids)] = ids
            slots_nat[s0:s0 + len(ids)] = ids
            npad = g[e] * P - len(ids)
            if npad:
                fill = ids[0] if len(ids) else 0
                perm[s0 + len(ids):s0 + g[e] * P] = fill
        slotnat.append(slots_nat)

        m = dict(common)
        m["xs"] = np.ascontiguousarray(x[c * T + perm])
        vsi = np.where(slots_nat >= 0, slots_nat, T).astype(np.int32)
        gg = np.where(slots_nat >= 0, slots_nat + (c % 2) * T, 0).astype(
            np.int32)
        csti = np.empty((P, 2 * K), np.int32)
        csti[:, 0:K] = vsi.reshape(K, P).T
        csti[:, K:2 * K] = gg.reshape(K, P).T
        m["csti"] = np.ascontiguousarray(csti)
        in_maps.append(m)
    proj_b_imm = float(np.asarray(inputs["proj_b"], f).reshape(-1)[0])
    return in_maps, proj_b_imm, g, slotnat


def _run(inputs, debug=False, trace=False):
    in_maps, proj_b_imm, g, slotnat = _prep_inputs(inputs)
    nc = build(proj_b_imm, g, debug=debug)
    res = run_bass_kernel_spmd(nc, in_maps, core_ids=list(range(NCORE)),
                               trace=trace)
    out = np.empty((B * N, D), np.float32)
    for c in range(NCORE):
        r = np.asarray(res.results[c]["out"])
        sn = slotnat[c]
        mask = sn >= 0
        out[c * T + sn[mask]] = r[mask]
    return out.reshape(B, N, D).astype(np.float32), res


def kernel(**inputs):
    out, _ = _run(inputs)
    return out


# revision 28
# speedup vs baseline: 1.1865x; 1.1865x over previous
"""Trainium2 Bass kernel: LayerNorm -> top-1 MoE -> v = clip(moe @ proj_w + b, +-3)
-> tridiagonal Green's-function diagonal via chunked Mobius scan
-> out = moe + bk*(spec @ out_w + out_b).

Sharding: data-parallel over flattened tokens (B*N = 8192) across 8 cores, 1024
tokens each (cores 2b/2b+1 own the halves of batch row b).

Top-1 routing is resolved on the HOST (fp32 LN+gate+argmax in numpy, matching
the reference's fp32 argmax): each core's tokens are sorted by expert into
per-expert column groups padded to 128-token tiles (group widths uniform
across cores => single SPMD program).  The device evaluates ONLY the selected
expert per token (~11/32 of the dense matmul work).  Per-token top softmax
prob is computed on device (1/sum(exp)).

Layout plumbing (all data-dependent indices are INPUTS, so the single SPMD
program serves all cores):
 - per-slot v is scattered to natural sequence order with per-tile indirect
   DMAs (overlapped with later groups' matmuls), pair-AllGathered, scanned.
 - G is written to DRAM packed (re,im) contiguously and gathered per slot
   with indirect DMAs.
 - weight/x/const DMAs are batched into few large transfers (the Sync
   sequencer costs ~0.6us per dma_start dispatch).
"""
import numpy as np
import ml_dtypes
_BF16NP = ml_dtypes.bfloat16
import concourse.bacc as bacc
import concourse.bass as bass
import concourse.mybir as mybir
from concourse.tile import TileContext
from concourse.bass_utils import run_bass_kernel_spmd
from concourse.alu_op_type import AluOpType

F32 = mybir.dt.float32
I32 = mybir.dt.int32
BF16 = mybir.dt.bfloat16
AF = mybir.ActivationFunctionType
AX = mybir.AxisListType
MULT, ADD, SUB = AluOpType.mult, AluOpType.add, AluOpType.subtract
MAXOP, MINOP = AluOpType.max, AluOpType.min

B, N, D, E = 4, 2048, 512, 4
H = 4 * D
P = 128
T = 1024          # real tokens per core
NCORE = 8

# cstA column offsets
CA_GAMMA, CA_BETA, CA_PROJW = 0, 512, 1024
CA_B2B, CA_B1C, CA_GWS, CA_GATEB, CA_IDM = 1536, 3584, 3648, 3664, 3668
CA_W = 3796
# cstB column offsets
CB_SHT, CB_JMAT, CB_W0, CB_W1O, CB_OUTB = 0, 896, 1024, 1536, 2048
CB_W = 2560


def build(proj_b_imm, g, debug=False):
    K = int(sum(g))               # token tiles per core (padded slot space)
    TS = K * P                    # slots per core
    S = np.concatenate([[0], np.cumsum(g)]).astype(int)  # tile starts/group

    nc = bacc.Bacc()
    dt = nc.dram_tensor
    xs = dt("xs", [TS, D], F32, kind="ExternalInput")
    cstA = dt("cstA", [P, CA_W], F32, kind="ExternalInput")
    cstB = dt("cstB", [P, CB_W], F32, kind="ExternalInput")
    csti = dt("csti", [P, 2 * K], I32, kind="ExternalInput")
    w1f = dt("w1f", [E * D, H], BF16, kind="ExternalInput")
    w2f = dt("w2f", [E * H, D], BF16, kind="ExternalInput")
    out = dt("out", [TS, D], F32, kind="ExternalOutput")
    # indirect-DMA targets need offset-0 APs -> standalone internal tensors
    vscat = dt("vscat", [T + 1, 1], F32, kind="Internal")
    grdp = dt("grdp", [N, 2], F32, kind="Internal")

    with TileContext(nc) as tc:
        with (tc.tile_pool(name="cst", bufs=1) as cst,
              tc.tile_pool(name="big", bufs=1) as big,
              tc.tile_pool(name="ht2", bufs=1) as ht2,
              tc.tile_pool(name="ln", bufs=1) as ln,
              tc.tile_pool(name="str", bufs=2) as strm,
              tc.tile_pool(name="rot", bufs=2) as rot,
              tc.tile_pool(name="sml", bufs=4) as sml,
              tc.tile_pool(name="ps", bufs=8, space="PSUM") as psp,
              tc.tile_pool(name="dr", bufs=1, space="DRAM") as dr):

            def tt(o, a, b, op):
                nc.vector.tensor_tensor(out=o, in0=a, in1=b, op=op)

            def gtt(o, a, b, op):
                nc.gpsimd.tensor_tensor(out=o, in0=a, in1=b, op=op)

            def ts(o, a, s1, s2, op0, op1=None):
                if op1 is None:
                    nc.vector.tensor_scalar(out=o, in0=a, scalar1=s1,
                                            scalar2=None, op0=op0)
                else:
                    nc.vector.tensor_scalar(out=o, in0=a, scalar1=s1,
                                            scalar2=s2, op0=op0, op1=op1)

            def gts(o, a, s1, s2, op0, op1=None):
                if op1 is None:
                    nc.gpsimd.tensor_scalar(out=o, in0=a, scalar1=s1,
                                            scalar2=None, op0=op0)
                else:
                    nc.gpsimd.tensor_scalar(out=o, in0=a, scalar1=s1,
                                            scalar2=s2, op0=op0, op1=op1)

            def stt(o, a, s, b, op0, op1):
                nc.vector.scalar_tensor_tensor(out=o, in0=a, scalar=s, in1=b,
                                               op0=op0, op1=op1)

            def gstt(o, a, s, b, op0, op1):
                nc.gpsimd.scalar_tensor_tensor(out=o, in0=a, scalar=s, in1=b,
                                               op0=op0, op1=op1)

            def cp(o, a):
                nc.vector.tensor_copy(out=o, in_=a)

            def gcp(o, a):
                nc.gpsimd.tensor_copy(out=o, in_=a)

            # ---- x per group (one DMA each), then packed constants ----
            xbig = []
            for e in range(E):
                ge = int(g[e])
                if ge == 0:
                    xbig.append(None)
                    continue
                xb = big.tile([P, ge * D], F32, tag=f"xbig{e}", name=f"xbig{e}")
                nc.sync.dma_start(
                    out=xb.rearrange("p (q j) -> p q j", q=ge),
                    in_=xs[S[e] * P:(S[e] + ge) * P, :].rearrange(
                        "(q p) j -> p q j", p=P))
                xbig.append(xb)

            cA = cst.tile([P, CA_W], F32, tag="cA")
            nc.sync.dma_start(out=cA, in_=cstA[:])
            cB = cst.tile([P, CB_W], F32, tag="cB")
            nc.sync.dma_start(out=cB, in_=cstB[:])
            cI = cst.tile([P, 2 * K], I32, tag="cI")
            nc.sync.dma_start(out=cI, in_=csti[:])

            gammat = cA[:, CA_GAMMA:CA_GAMMA + D]
            betat = cA[:, CA_BETA:CA_BETA + D]
            projwt = cA[:, CA_PROJW:CA_PROJW + D]
            b2bt = cA[:, CA_B2B:CA_B2B + E * D]
            b1ct = cA[:, CA_B1C:CA_B1C + 64]
            gwst = cA[:, CA_GWS:CA_GWS + 16]
            gatebt = cA[:, CA_GATEB:CA_GATEB + E]
            idt = cA[:, CA_IDM:CA_IDM + P]
            sht = cB[:, CB_SHT:CB_SHT + 7 * P]
            jmt = cB[:, CB_JMAT:CB_JMAT + P]
            w0t = cB[:, CB_W0:CB_W0 + D]
            w1ot = cB[:, CB_W1O:CB_W1O + D]
            outbt = cB[:, CB_OUTB:CB_OUTB + D]
            vsixt = cI[:, 0:K]
            ggixt = cI[:, K:2 * K]

            # warm-up collective: establish CC channels early
            wup = dr.tile([16], F32, name="wup", tag="wup")
            wug = dr.tile([32], F32, name="wug", tag="wug")
            wuz = sml.tile([1, 16], F32, tag="wuz")
            nc.vector.memset(wuz[:], 0.0)
            nc.sync.dma_start(out=wup[0:16], in_=wuz[0:1, :])
            nc.gpsimd.collective_compute(
                "AllGather", AluOpType.bypass,
                replica_groups=[[0, 1], [2, 3], [4, 5], [6, 7]],
                ins=[wup.opt()], outs=[wug.opt()])

            # per-group transposed activations (fp32 for gate, bf16 for mm1)
            xnTf = []
            xnTr = []
            for e in range(E):
                W = 128 * int(g[e])
                if W == 0:
                    xnTf.append(None)
                    xnTr.append(None)
                    continue
                xnTf.append(big.tile([P, 4 * W], F32, tag=f"xnTf{e}",
                                     name=f"xnTf{e}"))
                xnTr.append(big.tile([P, 4 * W], BF16, tag=f"xnTr{e}",
                                     name=f"xnTr{e}"))
            moe = big.tile([P, K * D], F32, tag="moe")   # = moe + out_b*bk
            ptall = sml.tile([P, K], F32, tag="ptall")
            vsb = sml.tile([P, K], F32, tag="vsb")
            gslot = sml.tile([P, 2 * K], F32, tag="gslot")

            vrow = dr.tile([2 * (T + 1)], F32, name="vrow", tag="vrow")

            IOX = bass.IndirectOffsetOnAxis

            # ========== fused stage A (LN/gate/transpose) + stage B per group
            for e in range(E):
                ge = int(g[e])
                if ge == 0:
                    continue
                W = 128 * ge
                # ---- stage A for this group's tiles ----
                for q in range(ge):
                    gt = S[e] + q
                    xt = xbig[e][:, q * D:(q + 1) * D]
                    musum = sml.tile([P, 1], F32, tag="musum")
                    nc.vector.tensor_reduce(out=musum, in_=xt, axis=AX.X,
                                            op=ADD)
                    mu = sml.tile([P, 1], F32, tag="mu")
                    ts(mu, musum, 1.0 / D, None, MULT)
                    mneg = sml.tile([P, 1], F32, tag="mnegl")
                    ts(mneg, musum, -1.0 / D, None, MULT)
                    scr = ln.tile([P, D], F32, tag="scr")
                    varsum = sml.tile([P, 1], F32, tag="varsum")
                    nc.scalar.activation(out=scr, in_=xt, func=AF.Square,
                                         bias=mneg[:, 0:1], scale=1.0,
                                         accum_out=varsum[:, 0:1])
                    vtmp = sml.tile([P, 1], F32, tag="vtmp")
                    ts(vtmp, varsum, 1.0 / D, 1e-5, MULT, ADD)
                    vsq = sml.tile([P, 1], F32, tag="vsq")
                    nc.scalar.activation(out=vsq, in_=vtmp, func=AF.Sqrt)
                    rstd = sml.tile([P, 1], F32, tag="rstd")
                    nc.vector.reciprocal(out=rstd, in_=vsq)
                    xn = ln.tile([P, D], F32, tag="xn")
                    ts(xn, xt, mu[:, 0:1], rstd[:, 0:1], SUB, MULT)
                    xng = ln.tile([P, D], F32, tag="xng")
                    tt(xng, xn, gammat, MULT)
                    xnf = ln.tile([P, D], F32, tag="xnf")
                    tt(xnf, xng, betat, ADD)
                    for db in range(4):
                        pst = psp.tile([P, P], F32, tag="mm")
                        nc.tensor.transpose(pst[:], xnf[:, db * P:(db + 1) * P],
                                            idt)
                        cp(xnTf[e][:, db * W + q * P: db * W + (q + 1) * P],
                           pst[:])
                    # gate logits -> top prob (softmax denominator reciprocal)
                    psg = psp.tile([P, E], F32, tag="mm")
                    for db in range(4):
                        nc.tensor.matmul(
                            out=psg,
                            lhsT=xnTf[e][:, db * W + q * P: db * W + (q + 1) * P],
                            rhs=gwst[:, db * E:(db + 1) * E],
                            start=(db == 0), stop=(db == 3))
                    lg = sml.tile([P, E], F32, tag="lg")
                    stt(lg, psg, 1.0, gatebt, MULT, ADD)
                    mx = sml.tile([P, 1], F32, tag="mx")
                    nc.vector.tensor_reduce(out=mx, in_=lg, axis=AX.X, op=MAXOP)
                    mxneg = sml.tile([P, 1], F32, tag="mxneg")
                    ts(mxneg, mx, -1.0, None, MULT)
                    el = sml.tile([P, E], F32, tag="el")
                    ssum = sml.tile([P, 1], F32, tag="ssum")
                    nc.scalar.activation(out=el, in_=lg, func=AF.Exp,
                                         bias=mxneg[:, 0:1], scale=1.0,
                                         accum_out=ssum[:, 0:1])
                    nc.vector.reciprocal(out=ptall[:, gt:gt + 1], in_=ssum)
                    xv_in = xnTf[e].rearrange("p (db t) -> p db t", db=4)[
                        :, :, q * P:(q + 1) * P]
                    xv_out = xnTr[e].rearrange("p (db t) -> p db t", db=4)[
                        :, :, q * P:(q + 1) * P]
                    gcp(xv_out, xv_in)

                # ---- stage B: selected-expert MoE for this group ----
                # weights fetched in 4-hc blocks (one dma_start per block)
                hT = ht2.tile([P, 16 * 512], BF16, tag=f"hT{e % 2}",
                              name=f"hT{e}")
                for hb in range(4):
                    w1blk = strm.tile([P, 4 * D], BF16, tag="w1blk")
                    nc.sync.dma_start(
                        out=w1blk.rearrange("p (db j) -> p db j", db=4),
                        in_=w1f[e * D:(e + 1) * D,
                                hb * 4 * P:(hb + 1) * 4 * P].rearrange(
                                    "(db p) j -> p db j", p=P))
                    for hq in range(4):
                        hc = hb * 4 + hq
                        psh = psp.tile([P, W], F32, tag="mm",
                                       name=f"psh{e}_{hc}")
                        for db in range(4):
                            nc.tensor.matmul(
                                out=psh,
                                lhsT=w1blk[:, db * 4 * P + hq * P:
                                           db * 4 * P + (hq + 1) * P],
                                rhs=xnTr[e][:, db * W:(db + 1) * W],
                                start=(db == 0), stop=(db == 3))
                        nc.scalar.activation(
                            out=hT[:, hc * W:(hc + 1) * W],
                            in_=psh, func=AF.Gelu_apprx_tanh,
                            bias=b1ct[:, e * 16 + hc: e * 16 + hc + 1],
                            scale=1.0)
                pso = [psp.tile([P, D], F32, tag="mm", name=f"pso{e}_{i}")
                       for i in range(ge)]
                for hb in range(4):
                    w2blk = strm.tile([P, 4 * D], BF16, tag="w2blk")
                    nc.sync.dma_start(
                        out=w2blk.rearrange("p (hc j) -> p hc j", hc=4),
                        in_=w2f[e * H + hb * 4 * P:
                                e * H + (hb + 1) * 4 * P, :].rearrange(
                                    "(hc p) j -> p hc j", p=P))
                    for hq in range(4):
                        hc = hb * 4 + hq
                        for q in range(ge):
                            nc.tensor.matmul(
                                out=pso[q],
                                lhsT=hT[:, hc * W + q * P: hc * W + (q + 1) * P],
                                rhs=w2blk[:, hq * D:(hq + 1) * D],
                                start=(hc == 0), stop=(hc == 15))
                for q in range(ge):
                    gt = S[e] + q
                    mslice = moe[:, gt * D:(gt + 1) * D]
                    pt_ap = ptall[:, gt:gt + 1]
                    if gt % 2 == 0:
                        eo = rot.tile([P, D], F32, tag="wrk")
                        stt(eo, pso[q], 1.0, b2bt[:, e * D:(e + 1) * D],
                            MULT, ADD)
                        # v from eo: (sum(eo*projw))*ptop + proj_b, clipped
                        tv = rot.tile([P, D], F32, tag="wrk")
                        tt(tv, eo, projwt, MULT)
                        vs = sml.tile([P, 1], F32, tag="vs")
                        nc.vector.tensor_reduce(out=vs, in_=tv,
                                                axis=AX.X, op=ADD)
                        vt1 = sml.tile([P, 1], F32, tag="vt1")
                        ts(vt1, vs, pt_ap, proj_b_imm, MULT, ADD)
                        ts(vsb[:, gt:gt + 1], vt1, 3.0, -3.0, MINOP, MAXOP)
                        # moe slice with out_b*bk folded in
                        stt(mslice, eo, pt_ap, outbt, MULT, ADD)
                    else:
                        eo = rot.tile([P, D], F32, tag="gwrk")
                        stt(eo, pso[q], 1.0, b2bt[:, e * D:(e + 1) * D],
                            MULT, ADD)
                        tv = rot.tile([P, D], F32, tag="gwrk")
                        gtt(tv, eo, projwt, MULT)
                        scr2 = rot.tile([P, D], F32, tag="swrk")
                        vsg = sml.tile([P, 1], F32, tag="vsg")
                        nc.scalar.activation(out=scr2, in_=tv,
                                             func=AF.Copy,
                                             accum_out=vsg[:, 0:1])
                        vt1g = sml.tile([P, 1], F32, tag="vt1g")
                        ts(vt1g, vsg, pt_ap, proj_b_imm, MULT, ADD)
                        ts(vsb[:, gt:gt + 1], vt1g, 3.0, -3.0, MINOP, MAXOP)
                        gtt(mslice, eo, pt_ap.broadcast_to([P, D]), MULT)
                        gtt(mslice, mslice, outbt, ADD)
                    # scatter this tile's v into natural positions now so the
                    # DMA overlaps the remaining groups' compute
                    nc.gpsimd.indirect_dma_start(
                        out=vscat[:], out_offset=IOX(ap=vsixt[:, gt:gt + 1],
                                                     axis=0),
                        in_=vsb[:, gt:gt + 1], in_offset=None)

            # ================= stage D: pair AllGather + scan inputs ==========
            nc.gpsimd.collective_compute(
                "AllGather", AluOpType.bypass,
                replica_groups=[[0, 1], [2, 3], [4, 5], [6, 7]],
                ins=[vscat[:].opt()], outs=[vrow.opt()])
            av = sml.tile([P, 16], F32, tag="av")
            nc.sync.dma_start(out=av[0:64, :], in_=vrow[0:T])
            nc.sync.dma_start(out=av[64:128, :], in_=vrow[T + 1:2 * T + 1])
            arf = sml.tile([P, 16], F32, tag="arf")
            ts(arf, av, -1.0, 2.0, MULT, ADD)        # a_re = 2 - v
            psj = psp.tile([P, 16], F32, tag="mm")
            nc.tensor.matmul(out=psj, lhsT=jmt, rhs=arf[:], start=True,
                             stop=True)
            arb = sml.tile([P, 16], F32, tag="arb")
            cp(arb, psj[:, 15::-1])                  # a_re reversed seq

            # ================= stage E: Mobius scan ===========================
            wfr = sml.tile([P, 36], F32, tag="wfr")
            wfi = sml.tile([P, 36], F32, tag="wfi")
            wbr = sml.tile([P, 36], F32, tag="wbr")
            wbi = sml.tile([P, 36], F32, tag="wbi")
            nc.vector.memset(wfr[:, 0:1], 0.0)
            nc.vector.memset(wfr[:, 1:2], 1.0)
            nc.vector.memset(wfr[:, 2:3], 1.0)
            nc.vector.memset(wfr[:, 3:4], 0.0)
            nc.vector.memset(wfi[:, 0:4], 0.0)
            nc.gpsimd.memset(wbr[:, 0:1], 0.0)
            nc.gpsimd.memset(wbr[:, 1:2], 1.0)
            nc.gpsimd.memset(wbr[:, 2:3], 1.0)
            nc.gpsimd.memset(wbr[:, 3:4], 0.0)
            nc.gpsimd.memset(wbi[:, 0:4], 0.0)
            taf = sml.tile([P, 2], F32, tag="taf")
            tbf = sml.tile([P, 2], F32, tag="tbf")
            tab = sml.tile([P, 2], F32, tag="tab")
            tbb = sml.tile([P, 2], F32, tag="tbb")
            for t in range(16):
                s0 = slice(2 * t, 2 * t + 2)
                s1 = slice(2 * t + 2, 2 * t + 4)
                s2 = slice(2 * t + 4, 2 * t + 6)
                af = arf[:, t:t + 1]
                ab = arb[:, t:t + 1]
                ts(taf, wfr[:, s1], af, None, MULT)
                tt(tbf, taf, wfi[:, s1], SUB)
                tt(wfr[:, s2], tbf, wfr[:, s0], SUB)
                ts(taf, wfi[:, s1], af, None, MULT)
                tt(tbf, taf, wfr[:, s1], ADD)
                tt(wfi[:, s2], tbf, wfi[:, s0], SUB)
                gtt(tab, wbr[:, s1], ab.broadcast_to([P, 2]), MULT)
                gtt(tbb, tab, wbi[:, s1], SUB)
                gtt(wbr[:, s2], tbb, wbr[:, s0], SUB)
                gtt(tab, wbi[:, s1], ab.broadcast_to([P, 2]), MULT)
                gtt(tbb, tab, wbr[:, s1], ADD)
                gtt(wbi[:, s2], tbb, wbi[:, s0], SUB)
            q = sml.tile([P, 16], F32, tag="qa")
            cp(q[:, 0:4:2], wfr[:, 34:36])      # m00, m01 = block 17
            cp(q[:, 4:8:2], wfr[:, 32:34])      # m10, m11 = block 16
            cp(q[:, 8:12:2], wfi[:, 34:36])
            cp(q[:, 12:16:2], wfi[:, 32:34])
            gcp(q[:, 1:4:2], wbr[:, 34:36])
            gcp(q[:, 5:8:2], wbr[:, 32:34])
            gcp(q[:, 9:12:2], wbi[:, 34:36])
            gcp(q[:, 13:16:2], wbi[:, 32:34])

            rn1 = sml.tile([P, 2], F32, tag="rn1")
            rn2 = sml.tile([P, 2], F32, tag="rn2")
            rn3 = sml.tile([P, 2], F32, tag="rn3")

            def renorm(qq):
                tt(rn1, qq[:, 0:2], qq[:, 0:2], MULT)
                tt(rn2, qq[:, 8:10], qq[:, 8:10], MULT)
                tt(rn3, rn1, rn2, ADD)
                nc.scalar.activation(out=rn1, in_=rn3, func=AF.Sqrt)
                nc.vector.reciprocal(out=rn2, in_=rn1)
                ts(qq[:, 0:16:2], qq[:, 0:16:2], rn2[:, 0:1], None, MULT)
                gtt(qq[:, 1:16:2], qq[:, 1:16:2],
                    rn2[:, 1:2].broadcast_to([P, 8]), MULT)

            renorm(q)

            p1t = sml.tile([P, 16], F32, tag="p1t")
            p2t = sml.tile([P, 16], F32, tag="p2t")
            p3t = sml.tile([P, 16], F32, tag="p3t")
            p4t = sml.tile([P, 16], F32, tag="p4t")
            crt = sml.tile([P, 16], F32, tag="crt")
            cit = sml.tile([P, 16], F32, tag="cit")

            def qa_v(tile16, ri, ii):
                v = tile16[:, 8 * ri + 4 * ii: 8 * ri + 4 * ii + 4]
                v = v.rearrange("p (k d) -> p k d", k=2, d=2)
                return v.unsqueeze(1).broadcast_to([P, 2, 2, 2])

            def qb_v(tile16, ri):
                v = tile16[:, 8 * ri: 8 * ri + 8]
                return v.rearrange("p (k j d) -> p j k d", k=2, j=2, d=2)

            def pv(tile16, ii):
                return tile16[:, 8 * ii: 8 * ii + 8].rearrange(
                    "p (j k d) -> p j k d", j=2, k=2, d=2)

            for i, s in enumerate((1, 2, 4, 8, 16, 32, 64)):
                psq = psp.tile([P, 16], F32, tag="mm")
                nc.tensor.matmul(out=psq, lhsT=sht[:, i * P:(i + 1) * P],
                                 rhs=q[:], start=True, stop=True)
                qs = sml.tile([P, 16], F32, tag=("qsa" if i % 2 == 0 else "qsb"))
                cp(qs, psq[:])
                nc.vector.memset(qs[0:s, 0:2], 1.0)   # identity pad m00
                nc.vector.memset(qs[0:s, 6:8], 1.0)   # identity pad m11
                qbr, qbi = qb_v(qs, 0), qb_v(qs, 1)
                for ii in range(2):
                    tt(pv(p1t, ii), qa_v(q, 0, ii), qbr, MULT)
                    tt(pv(p2t, ii), qa_v(q, 1, ii), qbi, MULT)
                    gtt(pv(p3t, ii), qa_v(q, 0, ii), qbi, MULT)
                    gtt(pv(p4t, ii), qa_v(q, 1, ii), qbr, MULT)
                tt(crt, p1t, p2t, SUB)
                gtt(cit, p3t, p4t, ADD)
                qn = sml.tile([P, 16], F32, tag=("qb" if i % 2 == 0 else "qa"))
                crv = crt.rearrange("p (ak d) -> p ak d", ak=8, d=2)
                civ = cit.rearrange("p (ak d) -> p ak d", ak=8, d=2)
                tt(qn[:, 0:8].rearrange("p (a d) -> p a d", a=4),
                   crv[:, 0:8:2, :], crv[:, 1:8:2, :], ADD)
                gtt(qn[:, 8:16].rearrange("p (a d) -> p a d", a=4),
                    civ[:, 0:8:2, :], civ[:, 1:8:2, :], ADD)
                q = qn
                if i in (2, 5):
                    renorm(q)

            # L3: interior values from the saved L1 partials
            psq1 = psp.tile([P, 16], F32, tag="mm")
            nc.tensor.matmul(out=psq1, lhsT=sht[:, 0:P], rhs=q[:],
                             start=True, stop=True)
            nc.vector.memset(psq1[0:1, 0:2], 1.0)      # chunk0 start x = 1
            sq1 = sml.tile([P, 16], F32, tag="sq1")
            cp(sq1, psq1[:])
            xsr_f, xsi_f = sq1[:, 0:1], sq1[:, 8:9]
            ysr_f, ysi_f = sq1[:, 4:5], sq1[:, 12:13]
            xsr_b, xsi_b = sq1[:, 1:2], sq1[:, 9:10]
            ysr_b, ysi_b = sq1[:, 5:6], sq1[:, 13:14]
            m00r, m00i = wfr[:, 2:36:2], wfi[:, 2:36:2]   # 17 blocks (1..17)
            m01r, m01i = wfr[:, 3:36:2], wfi[:, 3:36:2]
            n00r, n00i = wbr[:, 2:36:2], wbi[:, 2:36:2]
            n01r, n01i = wbr[:, 3:36:2], wbi[:, 3:36:2]
            ra = sml.tile([P, 17], F32, tag="ra")
            rb = sml.tile([P, 17], F32, tag="rb")
            rc = sml.tile([P, 17], F32, tag="rc")
            rd = sml.tile([P, 17], F32, tag="rd")
            gra = sml.tile([P, 17], F32, tag="gra")
            grb = sml.tile([P, 17], F32, tag="grb")
            grc = sml.tile([P, 17], F32, tag="grc")
            grd2 = sml.tile([P, 17], F32, tag="grd2")
            pxr = sml.tile([P, 17], F32, tag="pxr")
            pxi = sml.tile([P, 17], F32, tag="pxi")
            pbr = sml.tile([P, 17], F32, tag="pbr")
            pbi = sml.tile([P, 17], F32, tag="pbi")
            ts(ra, m00r, xsr_f, None, MULT)
            ts(rb, m00i, xsi_f, None, MULT)
            tt(rc, ra, rb, SUB)
            ts(ra, m01r, ysr_f, None, MULT)
            ts(rb, m01i, ysi_f, None, MULT)
            tt(rd, ra, rb, SUB)
            tt(pxr, rc, rd, ADD)
            ts(ra, m00r, xsi_f, None, MULT)
            ts(rb, m00i, xsr_f, None, MULT)
            tt(rc, ra, rb, ADD)
            ts(ra, m01r, ysi_f, None, MULT)
            ts(rb, m01i, ysr_f, None, MULT)
            tt(rd, ra, rb, ADD)
            tt(pxi, rc, rd, ADD)

            def bc17(apv):
                return apv.broadcast_to([P, 17])
            gtt(gra, n00r, bc17(xsr_b), MULT)
            gtt(grb, n00i, bc17(xsi_b), MULT)
            gtt(grc, gra, grb, SUB)
            gtt(gra, n01r, bc17(ysr_b), MULT)
            gtt(grb, n01i, bc17(ysi_b), MULT)
            gtt(grd2, gra, grb, SUB)
            gtt(pbr, grc, grd2, ADD)
            gtt(gra, n00r, bc17(xsi_b), MULT)
            gtt(grb, n00i, bc17(xsr_b), MULT)
            gtt(grc, gra, grb, ADD)
            gtt(gra, n01r, bc17(ysi_b), MULT)
            gtt(grb, n01i, bc17(ysr_b), MULT)
            gtt(grd2, gra, grb, ADD)
            gtt(pbi, grc, grd2, ADD)

            psfr = psp.tile([P, 17], F32, tag="mm")
            nc.tensor.matmul(out=psfr, lhsT=jmt, rhs=pbr[:], start=True,
                             stop=True)
            psfi = psp.tile([P, 17], F32, tag="mm")
            nc.tensor.matmul(out=psfi, lhsT=jmt, rhs=pbi[:], start=True,
                             stop=True)
            sfr = sml.tile([P, 17], F32, tag="sfr")
            sfi = sml.tile([P, 17], F32, tag="sfi")
            cp(sfr, psfr[:])
            cp(sfi, psfi[:])

            uxr, uxi = pxr[:, 1:17], pxi[:, 1:17]
            uyr, uyi = pxr[:, 0:16], pxi[:, 0:16]
            wxr, wxi = sfr[:, 16:0:-1], sfi[:, 16:0:-1]
            wyr, wyi = sfr[:, 15::-1], sfi[:, 15::-1]

            def ctile(tag):
                return sml.tile([P, 16], F32, tag=tag, name=tag)

            sa, sb = ctile("sa"), ctile("sb")
            ga, gb = ctile("ga"), ctile("gb")
            nr_, ni_ = ctile("nr"), ctile("ni")
            t1r, t1i = ctile("t1r"), ctile("t1i")
            t2r, t2i = ctile("t2r"), ctile("t2i")
            t3r, t3i = ctile("t3r"), ctile("t3i")
            drt, dit = ctile("drt"), ctile("dit")
            magt, invt = ctile("magt"), ctile("invt")
            gr, gi = ctile("gr"), ctile("gi")

            def cmul(or_, oi_, xr_, xi_, yr_, yi_):
                tt(sa, xr_, yr_, MULT)
                tt(sb, xi_, yi_, MULT)
                tt(or_, sa, sb, SUB)
                tt(sa, xr_, yi_, MULT)
                tt(sb, xi_, yr_, MULT)
                tt(oi_, sa, sb, ADD)

            def gcmul(or_, oi_, xr_, xi_, yr_, yi_):
                gtt(ga, xr_, yr_, MULT)
                gtt(gb, xi_, yi_, MULT)
                gtt(or_, ga, gb, SUB)
                gtt(ga, xr_, yi_, MULT)
                gtt(gb, xi_, yr_, MULT)
                gtt(oi_, ga, gb, ADD)

            cmul(nr_, ni_, uyr, uyi, wyr, wyi)       # num = Uy*Wy
            gcmul(t1r, t1i, uxr, uxi, wyr, wyi)
            cmul(t2r, t2i, wxr, wxi, uyr, uyi)
            gtt(ga, arf, nr_, MULT)                  # t3 = a*num, a = arf + 1j
            gtt(t3r, ga, ni_, SUB)
            gtt(gb, arf, ni_, MULT)
            gtt(t3i, gb, nr_, ADD)
            tt(sa, t1r, t2r, ADD)
            tt(drt, sa, t3r, SUB)
            tt(sb, t1i, t2i, ADD)
            tt(dit, sb, t3i, SUB)
            tt(sa, drt, drt, MULT)
            tt(sb, dit, dit, MULT)
            tt(magt, sa, sb, ADD)
            nc.vector.reciprocal(out=invt, in_=magt)
            tt(sa, nr_, drt, MULT)
            tt(sb, ni_, dit, MULT)
            tt(gr, sa, sb, ADD)
            tt(gr, gr, invt, MULT)
            gtt(ga, ni_, drt, MULT)
            gtt(gb, nr_, dit, MULT)
            gtt(gi, ga, gb, SUB)
            gtt(gi, gi, invt, MULT)

            # ============ stage F: G -> packed DRAM (contiguous) -> gather ====
            gpair = sml.tile([P, 32], F32, tag="gpair")
            cp(gpair.rearrange("p (j two) -> p j two", two=2)[:, :, 0], gr[:])
            cp(gpair.rearrange("p (j two) -> p j two", two=2)[:, :, 1], gi[:])
            nc.sync.dma_start(
                out=grdp[:, :].rearrange("(p j) two -> p (j two)", p=P),
                in_=gpair[:])
            for t in range(K):
                nc.gpsimd.indirect_dma_start(
                    out=gslot[:, 2 * t:2 * t + 2], out_offset=None,
                    in_=grdp[:],
                    in_offset=IOX(ap=ggixt[:, t:t + 1], axis=0))

            # ================= stage G: final combine (2 ops/tile) ============
            for t in range(K):
                acc = rot.tile([P, D], F32, tag="wrk")
                stt(acc, w0t, gslot[:, 2 * t:2 * t + 1],
                    moe[:, t * D:(t + 1) * D], MULT, ADD)
                ott = rot.tile([P, D], F32, tag="wrk")
                stt(ott, w1ot, gslot[:, 2 * t + 1:2 * t + 2], acc, MULT, ADD)
                nc.sync.dma_start(out=out[t * P:(t + 1) * P, :], in_=ott)
    nc.finalize()
    return nc


def _prep_inputs(inputs):
    f = np.float32
    x = np.ascontiguousarray(np.asarray(inputs["x"], f).reshape(B * N, D))
    gamma = np.asarray(inputs["ln_gamma"], f)
    beta = np.asarray(inputs["ln_beta"], f)
    gate_w = np.asarray(inputs["gate_w"], f)
    gate_b = np.asarray(inputs["gate_b"], f)
    w1 = np.asarray(inputs["w1"], f)
    b1 = np.asarray(inputs["b1"], f)
    w2 = np.asarray(inputs["w2"], f)
    b2 = np.asarray(inputs["b2"], f)
    proj_w = np.asarray(inputs["proj_w"], f)[:, 0]
    out_w = np.asarray(inputs["out_w"], f)
    out_b = np.asarray(inputs["out_b"], f)
    bk = f(np.asarray(inputs["bk_scale"], f).reshape(-1)[0])

    # ---- host routing (argmax of gate logits over LN'd x, fp32) ----
    mu = x.mean(-1, keepdims=True, dtype=np.float32)
    xc = x - mu
    var = np.mean(xc * xc, axis=-1, keepdims=True, dtype=np.float32)
    xn = xc / np.sqrt(var + np.float32(1e-5)) * gamma + beta
    lg = xn @ gate_w + gate_b
    eid = lg.argmax(-1)                                    # (8192,)

    cnt = np.zeros((NCORE, E), np.int64)
    for c in range(NCORE):
        cnt[c] = np.bincount(eid[c * T:(c + 1) * T], minlength=E)
    g = np.ceil(cnt.max(axis=0) / P).astype(int)           # tiles per expert
    K = int(g.sum())
    TS = K * P
    S = np.concatenate([[0], np.cumsum(g)]).astype(int)

    def bcast(v, w):
        return np.ascontiguousarray(np.broadcast_to(v.astype(f), (P, w)))

    cstA = np.zeros((P, CA_W), f)
    cstA[:, CA_GAMMA:CA_GAMMA + D] = bcast(gamma, D)
    cstA[:, CA_BETA:CA_BETA + D] = bcast(beta, D)
    cstA[:, CA_PROJW:CA_PROJW + D] = bcast(proj_w, D)
    cstA[:, CA_B2B:CA_B2B + E * D] = bcast(b2.reshape(E * D), E * D)
    cstA[:, CA_B1C:CA_B1C + 64] = np.ascontiguousarray(
        b1.reshape(E, 16, P).transpose(2, 0, 1).reshape(P, 64))
    cstA[:, CA_GWS:CA_GWS + 16] = np.ascontiguousarray(
        gate_w.reshape(4, P, E).transpose(1, 0, 2).reshape(P, 16))
    cstA[:, CA_GATEB:CA_GATEB + E] = bcast(gate_b, E)
    cstA[:, CA_IDM:CA_IDM + P] = np.eye(P, dtype=f)

    cstB = np.zeros((P, CB_W), f)
    cstB[:, CB_SHT:CB_SHT + 7 * P] = np.concatenate(
        [np.eye(P, k=s, dtype=f) for s in (1, 2, 4, 8, 16, 32, 64)], axis=1)
    cstB[:, CB_JMAT:CB_JMAT + P] = np.eye(P, dtype=f)[::-1]
    cstB[:, CB_W0:CB_W0 + D] = bcast(out_w[0] * bk, D)
    cstB[:, CB_W1O:CB_W1O + D] = bcast(out_w[1] * bk, D)
    cstB[:, CB_OUTB:CB_OUTB + D] = bcast(out_b * bk, D)

    common = dict(
        cstA=cstA,
        cstB=cstB,
        w1f=np.ascontiguousarray(w1.reshape(E * D, H)).astype(_BF16NP),
        w2f=np.ascontiguousarray(w2.reshape(E * H, D)).astype(_BF16NP),
    )
    in_maps = []
    slotnat = []                     # per core: slot -> local natural (-1 pad)
    for c in range(NCORE):
        ec = eid[c * T:(c + 1) * T]
        perm = np.full(TS, -1, np.int64)
        slots_nat = np.full(TS, -1, np.int64)
        for e in range(E):
            ids = np.nonzero(ec == e)[0]
            if g[e] == 0:
                continue
            s0 = S[e] * P
            perm[s0:s0 + len(ids)] = ids
            slots_nat[s0:s0 + len(ids)] = ids
            npad = g[e] * P - len(ids)
            if npad:
                fill = ids[0] if len(ids) else 0
                perm[s0 + len(ids):s0 + g[e] * P] = fill
        slotnat.append(slots_nat)

        m = dict(common)
        m["xs"] = np.ascontiguousarray(x[c * T + perm])
        vsi = np.where(slots_nat >= 0, slots_nat, T).astype(np.int32)
        gg = np.where(slots_nat >= 0, slots_nat + (c % 2) * T, 0).astype(
            np.int32)
        csti = np.empty((P, 2 * K), np.int32)
        csti[:, 0:K] = vsi.reshape(K, P).T
        csti[:, K:2 * K] = gg.reshape(K, P).T
        m["csti"] = np.ascontiguousarray(csti)
        in_maps.append(m)
    proj_b_imm = float(np.asarray(inputs["proj_b"], f).reshape(-1)[0])
    return in_maps, proj_b_imm, g, slotnat


def _run(inputs, debug=False, trace=False):
    in_maps, proj_b_imm, g, slotnat = _prep_inputs(inputs)
    nc = build(proj_b_imm, g, debug=debug)
    res = run_bass_kernel_spmd(nc, in_maps, core_ids=list(range(NCORE)),
                               trace=trace)
    out = np.empty((B * N, D), np.float32)
    for c in range(NCORE):
        r = np.asarray(res.results[c]["out"])
        sn = slotnat[c]
        mask = sn >= 0
        out[c * T + sn[mask]] = r[mask]
    return out.reshape(B, N, D).astype(np.float32), res


def kernel(**inputs):
    out, _ = _run(inputs)
    return out
